# revision 28
# baseline (speedup 1.0000x reference)
"""Binary-tree gated-expert MoE kernel for 8 Trainium2 NeuronCores.

Reference computation (B=4096, D=2048, 4 levels, 1/2/4/8 experts):
    h = x
    for level l: h = relu(h @ Wl[eid_l] + bl[eid_l])
where eid_l is the l-bit prefix of the 3-bit leaf id built from
path_mask[:, 0:3].

Strategy: sibling-paired half-leaf dispatch.  Samples are grouped by
leaf; sibling leaves (2j, 2j+1) share their level-0..2 experts and
differ only at level 3.  Each of the two cores serving pair j takes
half of leaf 2j in PSUM column-chunk 0 and half of leaf 2j+1 in chunk
1.  Because the per-(jt,kt) matmul is issued per PSUM chunk anyway
(Bc > 512 needs two chunks), pointing chunk 1 at a second level-3
weight tile costs nothing on the Tensor engine, and both chunks stay
large enough (>= ~240 rows) that the ~97ns LDWEIGHTS pipeline stays
hidden under the matmuls.  This balances the per-core batch to
Bc = ceil(max_even_leaf/2) + ceil(max_odd_leaf/2) (~528) instead of
max_leaf (~544), with zero small-matmul splits.

Everything flows in bfloat16 (weights, x, inter-level activations and
the output, upconverted on host); PSUM accumulation stays fp32, so the
end-to-end rel-err vs the fp32 reference is ~5e-3, well inside the
2e-2 gate.  bf16 also halves HBM traffic and SBUF footprint.  Weights
stream HBM->SBUF per 512-feature group on a paced DMA chain (level 3
streams two expert matrices, prefetched during level 2); x streams on
the scalar queue so its issue latency overlaps the weight chain's.
"""

import math

import numpy as np
import ml_dtypes

from concourse import bacc, mybir, tile
from concourse.bass_utils import run_bass_kernel_spmd

D = 2048
KT = D // 128          # 16 contraction k-tiles
JT = D // 128          # 16 output-feature blocks
JG = 4                 # j-groups of 4 blocks (512 features) per W DMA
N_CORES = 8
N_LEVELS = 4
F32 = mybir.dt.float32
BF16 = mybir.dt.bfloat16
BF16_NP = ml_dtypes.bfloat16

NQ = 4                 # W DMA split: quarters of 4 k-tiles
PACE_WIN = 4           # max in-flight paced DMAs on the SP ring
WARM_N = 8             # PE p-state warmup matmuls
W_BUFS = 6             # weight tile buffering (deep for L3 dual stream)

_cache: dict = {}


def _build(c0: int, c1: int):
    """Build + compile the per-core Bass program.  PSUM chunk 0 holds
    ``c0`` columns (even-leaf half), chunk 1 ``c1`` columns (odd-leaf
    half); both use one weight stream for levels 0-2 and separate
    level-3 streams W3A / W3B."""
    key = (c0, c1)
    if key in _cache:
        return _cache[key]
    Bc = c0 + c1

    nc = bacc.Bacc("TRN2", target_bir_lowering=False, debug=False,
                   num_devices=N_CORES)

    # Weights arrive host-linearized as [JG, 128, KT*512]:
    # element (jg, p, kt, jc) = W[kt*128 + p, jg*512 + jc], so each DMA
    # reads long contiguous runs per partition.
    # x and out are host-linearized to the exact SBUF per-partition
    # layout [128, KT*Bc]: DMA runs are then len(ks)*Bc*2 bytes
    # contiguous per partition (2-8KB) instead of the 1KB runs a
    # [D, Bc] layout would give -- the early x stream runs at full
    # HBM rate instead of ~half.
    xT = nc.dram_tensor("xT", [128, KT * Bc], BF16, kind="ExternalInput")
    Wshape = [JG, 128, KT * 512]
    Ws = [nc.dram_tensor(f"W{l}", Wshape, BF16, kind="ExternalInput")
          for l in range(N_LEVELS - 1)]
    W3A = nc.dram_tensor("W3A", Wshape, BF16, kind="ExternalInput")
    W3B = nc.dram_tensor("W3B", Wshape, BF16, kind="ExternalInput")
    bias = nc.dram_tensor("bias", [N_LEVELS + 1, D], F32,
                          kind="ExternalInput")
    out = nc.dram_tensor("out", [128, JT * Bc], BF16, kind="ExternalOutput")

    xTv = xT.rearrange("p (kt b) -> p kt b", b=Bc)
    outv = out.rearrange("p (jt b) -> p jt b", b=Bc)
    bv = bias.rearrange("l (jt p) -> p l jt", p=128)
    KQ = KT // NQ               # k-tiles per quarter
    QW = KQ * 512               # W free-dim elements per quarter

    csl = (slice(0, c0), slice(c0, Bc))

    with tile.TileContext(nc) as tc:
        with (
            tc.tile_pool(name="acts", bufs=1) as acts,
            tc.tile_pool(name="w", bufs=W_BUFS) as wpool,
            tc.tile_pool(name="ps", bufs=8, space="PSUM") as ps,
            tc.tile_pool(name="misc", bufs=1) as misc,
        ):
            actA = acts.tile([128, KT, Bc], BF16, tag="A")
            actB = acts.tile([128, KT, Bc], BF16, tag="B")
            btile = misc.tile([128, N_LEVELS + 1, JT], F32)
            nc.scalar.dma_start(btile[:], bv)

            # Warm the PE HAM clock gate during the DMA lead-in:
            # throwaway matmuls on a zeroed tile so the first real
            # matmul runs at full clock instead of ramping on real work.
            warm = misc.tile([128, 512], BF16)
            nc.gpsimd.memset(warm[:], 0.0)
            # one accumulation group: back-to-back matmuls into the
            # same PSUM bank have no start/stop turnaround bubble
            wacc = ps.tile([128, 512], F32, tag="ps", name="wacc")
            for i in range(WARM_N):
                nc.tensor.matmul(wacc[:], warm[:, :128], warm[:],
                                 start=(i == 0), stop=(i == WARM_N - 1))

            # Weight DMAs go on the SP ring, chained so a bounded
            # number are in flight.  The HW SDMA engines round-robin
            # packets across every queued transfer, so an unbounded
            # backlog makes every transfer finish near the end; a short
            # chain keeps completion order = consumption order with the
            # stream still running at full HBM rate.
            paced = []

            def paced_dma(dst_ap, src_ap):
                h = nc.sync.dma_start(dst_ap, src_ap)
                n = len(paced)
                win = 2 if n < 4 else PACE_WIN
                if n >= win:
                    tile.add_dep_helper(h.ins, paced[-win].ins,
                                        reason="dma pacing chain")
                paced.append(h)
                return h

            # x k-pieces ride the same paced chain, interleaved with
            # the first W pieces in consumption-priority order: the
            # first matmuls need only k-tiles 0-1 of x and W.
            pend_x = [slice(KQ // 2, KQ)]
            pend_x += [slice(q * KQ, (q + 1) * KQ) for q in range(1, NQ)]

            def dma_x_piece():
                ks = pend_x.pop(0)
                paced_dma(actA[:, ks, :], xTv[:, ks, :])

            def dma_w_tile(wt, src, first=False):
                wflat = wt.rearrange("p kt j -> p (kt j)")
                if first:
                    # halve the very first W piece and interleave the
                    # x stream between W pieces
                    h = QW // 2
                    paced_dma(actA[:, 0:KQ // 2, :], xTv[:, 0:KQ // 2, :])
                    paced_dma(wflat[:, 0:h], src[:, 0:h])
                    dma_x_piece()
                    paced_dma(wflat[:, h:QW], src[:, h:QW])
                else:
                    paced_dma(wflat[:, 0:QW], src[:, 0:QW])
                if pend_x:
                    dma_x_piece()
                for q in range(1, NQ):
                    paced_dma(wflat[:, q * QW:(q + 1) * QW],
                              src[:, q * QW:(q + 1) * QW])
                    if pend_x:
                        dma_x_piece()

            for l in range(N_LEVELS):
                src = actA if l % 2 == 0 else actB
                dst = actB if l % 2 == 0 else actA
                last = l == N_LEVELS - 1
                for jg in range(JG):
                    if not last:
                        wt = wpool.tile([128, KT, 4 * 128], BF16, tag="w")
                        dma_w_tile(wt, Ws[l][jg], first=(l == 0 and jg == 0))
                        wts = (wt, wt)
                    else:
                        wtA = wpool.tile([128, KT, 4 * 128], BF16, tag="w")
                        dma_w_tile(wtA, W3A[jg])
                        wtB = wpool.tile([128, KT, 4 * 128], BF16, tag="w")
                        dma_w_tile(wtB, W3B[jg])
                        wts = (wtA, wtB)
                    accs = [ps.tile([128, (c0, c1)[c]], F32, tag="ps",
                                    name="acc")
                            for c in range(2) for _ in range(4)]
                    for q in range(NQ):
                        for jj in range(4):
                            # Keep-warm fillers where the level-0 x+W
                            # stream is known to run behind the PE: a
                            # stall > ~3us re-gates the HAM clock to
                            # 1.2GHz and costs ~3us of half-speed
                            # matmuls on top of the stall itself.
                            # Fillers write into accs[jj] BEFORE its
                            # first real start=True matmul (q == 0
                            # only), which resets the bank, so they
                            # are correctness-neutral.
                            nfill = 0
                            if q == 0 and l == 0 and jg == 0 and jj > 0:
                                nfill = 3
                            elif q == 0 and l == 0 and jg == 1 and jj == 0:
                                nfill = 6
                            fa = accs[jj]
                            for _ in range(nfill):
                                nc.tensor.matmul(fa[:], warm[:, :128],
                                                 warm[:, :c0],
                                                 start=True, stop=True)
                            for kt in range(q * KQ, (q + 1) * KQ):
                                for c in range(2):
                                    nc.tensor.matmul(
                                        accs[c * 4 + jj][:],
                                        wts[c][:, kt,
                                               jj * 128:(jj + 1) * 128],
                                        src[:, kt, csl[c]],
                                        start=(kt == 0),
                                        stop=(kt == KT - 1),
                                    )
                    for c in range(2):
                        bslot = l if not last else (3 + c)
                        for jj in range(4):
                            jt = jg * 4 + jj
                            nc.scalar.activation(
                                dst[:, jt, csl[c]], accs[c * 4 + jj][:],
                                mybir.ActivationFunctionType.Relu,
                                bias=btile[:, bslot, jt:jt + 1],
                            )
                    if last:
                        # ship this jg's four feature blocks via SWDGE
                        # (GpSimd) so the store never head-of-line
                        # blocks the paced W chain.  Last jg goes out
                        # per (chunk, block) so the tail DMA is small.
                        if jg < JG - 1:
                            nc.gpsimd.dma_start(
                                outv[:, jg * 4:(jg + 1) * 4, :],
                                dst[:, jg * 4:(jg + 1) * 4, :])
                        else:
                            for c in range(2):
                                for jj in range(4):
                                    jt = jg * 4 + jj
                                    fin = (c == 1 and jj == 3)
                                    eng = nc.scalar if fin else nc.gpsimd
                                    eng.dma_start(outv[:, jt, csl[c]],
                                                  dst[:, jt, csl[c]])

    nc.compile()
    _cache[key] = nc
    return nc


def _linearize_w(W: np.ndarray) -> np.ndarray:
    """[D, D] -> bf16 [JG, 128, KT*512] with
    (jg, p, kt, jc) = W[kt*128+p, jg*512+jc]."""
    return np.ascontiguousarray(
        W.astype(BF16_NP).reshape(KT, 128, JG, 512)
        .transpose(2, 1, 0, 3).reshape(JG, 128, KT * 512))


def kernel(x, path_mask, W0, b0, W1, b1, W2, b2, W3, b3, _trace=False):
    x = np.asarray(x, dtype=np.float32)
    Wls = [np.asarray(W, dtype=np.float32) for W in (W0, W1, W2, W3)]
    bls = [np.asarray(b, dtype=np.float32) for b in (b0, b1, b2, b3)]
    B = x.shape[0]

    pm = np.asarray(path_mask)
    e3 = (pm[:, 0] * 4 + pm[:, 1] * 2 + pm[:, 2]).astype(np.int64)
    leaf_rows = [np.nonzero(e3 == e)[0] for e in range(8)]
    counts = np.array([len(r) for r in leaf_rows])

    # per-core chunk assignment: core 2j+k gets half k of leaf 2j in
    # chunk 0 and half k of leaf 2j+1 in chunk 1.
    halves = [None] * 8  # halves[leaf] = (rows_half0, rows_half1)
    for e in range(8):
        h = (counts[e] + 1) // 2
        halves[e] = (leaf_rows[e][:h], leaf_rows[e][h:])
    c0 = max(2, (max(len(halves[2 * j][0]) for j in range(4)) + 1) // 2 * 2)
    c1 = max(2, (max(len(halves[2 * j + 1][0]) for j in range(4)) + 1) // 2 * 2)
    # nseg > 1 only under extreme routing skew (a leaf with > 1024
    # rows); each extra segment re-runs the kernel on the overflow.
    nseg = max(1, math.ceil(c0 / 512), math.ceil(c1 / 512))
    c0 = min(c0, 512)
    c1 = min(c1, 512)
    Bc = c0 + c1
    nc = _build(c0, c1)

    xT_bf = np.ascontiguousarray(x.T.astype(BF16_NP))
    in_common = []
    for cid in range(N_CORES):
        j = cid // 2
        eids = (0, j >> 1, j)
        m = {f"W{l}": _linearize_w(Wls[l][eids[l]]) for l in range(3)}
        m["W3A"] = _linearize_w(Wls[3][2 * j])
        m["W3B"] = _linearize_w(Wls[3][2 * j + 1])
        m["bias"] = np.ascontiguousarray(np.stack(
            [bls[0][0], bls[1][j >> 1], bls[2][j],
             bls[3][2 * j], bls[3][2 * j + 1]]))
        in_common.append(m)

    core_groups = []
    for cid in range(N_CORES):
        j, k = cid // 2, cid % 2
        core_groups.append((halves[2 * j][k], halves[2 * j + 1][k]))

    out_full = np.zeros((B, D), dtype=np.float32)
    last_res = None
    for s in range(nseg):
        in_maps = []
        segs = []
        for cid in range(N_CORES):
            g0, g1 = core_groups[cid]
            g0 = g0[s * c0:(s + 1) * c0]
            g1 = g1[s * c1:(s + 1) * c1]
            segs.append((g0, g1))
            xTc = np.zeros((D, Bc), dtype=BF16_NP)
            xTc[:, :len(g0)] = xT_bf[:, g0]
            xTc[:, c0:c0 + len(g1)] = xT_bf[:, g1]
            # [D, Bc] -> SBUF-layout [128, KT*Bc]
            xlin = np.ascontiguousarray(
                xTc.reshape(KT, 128, Bc).transpose(1, 0, 2)
                .reshape(128, KT * Bc))
            in_maps.append({"xT": xlin, **in_common[cid]})
        res = run_bass_kernel_spmd(nc, in_maps, list(range(N_CORES)),
                                   trace=_trace)
        last_res = res
        for cid in range(N_CORES):
            g0, g1 = segs[cid]
            # SBUF-layout [128, JT*Bc] -> [D, Bc]
            o = res.results[cid]["out"].reshape(128, JT, Bc)
            o = o.transpose(1, 0, 2).reshape(D, Bc)
            out_full[g0] = o[:, :len(g0)].astype(np.float32).T
            out_full[g1] = o[:, c0:c0 + len(g1)].astype(np.float32).T
    if _trace:
        return out_full, last_res
    return out_full


# revision 29
# speedup vs baseline: 1.0229x; 1.0229x over previous
"""Binary-tree gated-expert MoE kernel for 8 Trainium2 NeuronCores.

Reference computation (B=4096, D=2048, 4 levels, 1/2/4/8 experts):
    h = x
    for level l: h = relu(h @ Wl[eid_l] + bl[eid_l])
where eid_l is the l-bit prefix of the 3-bit leaf id built from
path_mask[:, 0:3].

Strategy: sibling-paired half-leaf dispatch.  Samples are grouped by
leaf; sibling leaves (2j, 2j+1) share their level-0..2 experts and
differ only at level 3.  Each of the two cores serving pair j takes
half of leaf 2j in PSUM column-chunk 0 and half of leaf 2j+1 in chunk
1.  Because the per-(jt,kt) matmul is issued per PSUM chunk anyway
(Bc > 512 needs two chunks), pointing chunk 1 at a second level-3
weight tile costs nothing on the Tensor engine, and both chunks stay
large enough (>= ~240 rows) that the ~97ns LDWEIGHTS pipeline stays
hidden under the matmuls.  This balances the per-core batch to
Bc = ceil(max_even_leaf/2) + ceil(max_odd_leaf/2) (~528) instead of
max_leaf (~544), with zero small-matmul splits.

Everything flows in bfloat16 (weights, x, inter-level activations and
the output, upconverted on host); PSUM accumulation stays fp32, so the
end-to-end rel-err vs the fp32 reference is ~5e-3, well inside the
2e-2 gate.  bf16 also halves HBM traffic and SBUF footprint.  Weights
stream HBM->SBUF per 512-feature group on a paced DMA chain (level 3
streams two expert matrices, prefetched during level 2); x streams on
the scalar queue so its issue latency overlaps the weight chain's.
"""

import math

import numpy as np
import ml_dtypes

from concourse import bacc, mybir, tile
from concourse.bass_utils import run_bass_kernel_spmd

D = 2048
KT = D // 128          # 16 contraction k-tiles
JT = D // 128          # 16 output-feature blocks
JG = 4                 # j-groups of 4 blocks (512 features) per W DMA
N_CORES = 8
N_LEVELS = 4
F32 = mybir.dt.float32
BF16 = mybir.dt.bfloat16
BF16_NP = ml_dtypes.bfloat16

NQ = 4                 # W DMA split: quarters of 4 k-tiles
PACE_WIN = 4           # max in-flight paced DMAs on the SP ring
WARM_N = 8             # PE p-state warmup matmuls
W_BUFS = 6             # weight tile buffering (deep for L3 dual stream)

_cache: dict = {}


def _build(c0: int, c1: int):
    """Build + compile the per-core Bass program.  PSUM chunk 0 holds
    ``c0`` columns (even-leaf half), chunk 1 ``c1`` columns (odd-leaf
    half); both use one weight stream for levels 0-2 and separate
    level-3 streams W3A / W3B."""
    key = (c0, c1)
    if key in _cache:
        return _cache[key]
    Bc = c0 + c1

    nc = bacc.Bacc("TRN2", target_bir_lowering=False, debug=False,
                   num_devices=N_CORES)

    # Weights arrive host-linearized as [JG, 128, KT*512]:
    # element (jg, p, kt, jc) = W[kt*128 + p, jg*512 + jc], so each DMA
    # reads long contiguous runs per partition.
    # x and out are host-linearized to the exact SBUF per-partition
    # layout [128, KT*Bc]: DMA runs are then len(ks)*Bc*2 bytes
    # contiguous per partition (2-8KB) instead of the 1KB runs a
    # [D, Bc] layout would give -- the early x stream runs at full
    # HBM rate instead of ~half.
    xT = nc.dram_tensor("xT", [128, KT * Bc], BF16, kind="ExternalInput")
    Wshape = [JG, 128, KT * 512]
    Ws = [nc.dram_tensor(f"W{l}", Wshape, BF16, kind="ExternalInput")
          for l in range(N_LEVELS - 1)]
    W3A = nc.dram_tensor("W3A", Wshape, BF16, kind="ExternalInput")
    W3B = nc.dram_tensor("W3B", Wshape, BF16, kind="ExternalInput")
    bias = nc.dram_tensor("bias", [N_LEVELS + 1, D], F32,
                          kind="ExternalInput")
    out = nc.dram_tensor("out", [128, JT * Bc], BF16, kind="ExternalOutput")

    xTv = xT.rearrange("p (kt b) -> p kt b", b=Bc)
    outv = out.rearrange("p (jt b) -> p jt b", b=Bc)
    bv = bias.rearrange("l (jt p) -> p l jt", p=128)
    KQ = KT // NQ               # k-tiles per quarter
    QW = KQ * 512               # W free-dim elements per quarter

    csl = (slice(0, c0), slice(c0, Bc))

    with tile.TileContext(nc) as tc:
        with (
            tc.tile_pool(name="acts", bufs=1) as acts,
            tc.tile_pool(name="w", bufs=W_BUFS) as wpool,
            tc.tile_pool(name="ps", bufs=8, space="PSUM") as ps,
            tc.tile_pool(name="misc", bufs=1) as misc,
        ):
            actA = acts.tile([128, KT, Bc], BF16, tag="A")
            actB = acts.tile([128, KT, Bc], BF16, tag="B")
            btile = misc.tile([128, N_LEVELS + 1, JT], F32)
            nc.scalar.dma_start(btile[:], bv)

            # Warm the PE HAM clock gate during the DMA lead-in:
            # throwaway matmuls on a zeroed tile so the first real
            # matmul runs at full clock instead of ramping on real work.
            warm = misc.tile([128, 512], BF16)
            nc.gpsimd.memset(warm[:], 0.0)
            # one accumulation group: back-to-back matmuls into the
            # same PSUM bank have no start/stop turnaround bubble
            wacc = ps.tile([128, 512], F32, tag="ps", name="wacc")
            for i in range(WARM_N):
                nc.tensor.matmul(wacc[:], warm[:, :128], warm[:],
                                 start=(i == 0), stop=(i == WARM_N - 1))

            # Weight DMAs go on the SP ring, chained so a bounded
            # number are in flight.  The HW SDMA engines round-robin
            # packets across every queued transfer, so an unbounded
            # backlog makes every transfer finish near the end; a short
            # chain keeps completion order = consumption order with the
            # stream still running at full HBM rate.
            paced = []

            def paced_dma(dst_ap, src_ap):
                h = nc.sync.dma_start(dst_ap, src_ap)
                n = len(paced)
                win = 2 if n < 4 else PACE_WIN
                if n >= win:
                    tile.add_dep_helper(h.ins, paced[-win].ins,
                                        reason="dma pacing chain")
                paced.append(h)
                return h

            # x k-pieces ride the same paced chain, interleaved with
            # the first W pieces in consumption-priority order: the
            # first matmuls need only k-tiles 0-1 of x and W.
            pend_x = [slice(KQ // 2, KQ)]
            pend_x += [slice(q * KQ, (q + 1) * KQ) for q in range(1, NQ)]

            def dma_x_piece():
                ks = pend_x.pop(0)
                paced_dma(actA[:, ks, :], xTv[:, ks, :])

            def dma_w_tile(wt, src, first=False):
                wflat = wt.rearrange("p kt j -> p (kt j)")
                if first:
                    # halve the very first W piece and interleave the
                    # x stream between W pieces
                    h = QW // 2
                    paced_dma(actA[:, 0:KQ // 2, :], xTv[:, 0:KQ // 2, :])
                    paced_dma(wflat[:, 0:h], src[:, 0:h])
                    dma_x_piece()
                    paced_dma(wflat[:, h:QW], src[:, h:QW])
                else:
                    paced_dma(wflat[:, 0:QW], src[:, 0:QW])
                if pend_x:
                    dma_x_piece()
                for q in range(1, NQ):
                    paced_dma(wflat[:, q * QW:(q + 1) * QW],
                              src[:, q * QW:(q + 1) * QW])
                    if pend_x:
                        dma_x_piece()

            for l in range(N_LEVELS):
                src = actA if l % 2 == 0 else actB
                dst = actB if l % 2 == 0 else actA
                last = l == N_LEVELS - 1
                for jg in range(JG):
                    if not last:
                        wt = wpool.tile([128, KT, 4 * 128], BF16, tag="w")
                        dma_w_tile(wt, Ws[l][jg], first=(l == 0 and jg == 0))
                        wts = (wt, wt)
                    else:
                        wtA = wpool.tile([128, KT, 4 * 128], BF16, tag="w")
                        dma_w_tile(wtA, W3A[jg])
                        wtB = wpool.tile([128, KT, 4 * 128], BF16, tag="w")
                        dma_w_tile(wtB, W3B[jg])
                        wts = (wtA, wtB)
                    accs = [ps.tile([128, (c0, c1)[c]], F32, tag="ps",
                                    name="acc")
                            for c in range(2) for _ in range(4)]
                    for q in range(NQ):
                        for jj in range(4):
                            for kt in range(q * KQ, (q + 1) * KQ):
                                for c in range(2):
                                    nc.tensor.matmul(
                                        accs[c * 4 + jj][:],
                                        wts[c][:, kt,
                                               jj * 128:(jj + 1) * 128],
                                        src[:, kt, csl[c]],
                                        start=(kt == 0),
                                        stop=(kt == KT - 1),
                                    )
                    for c in range(2):
                        bslot = l if not last else (3 + c)
                        for jj in range(4):
                            jt = jg * 4 + jj
                            nc.scalar.activation(
                                dst[:, jt, csl[c]], accs[c * 4 + jj][:],
                                mybir.ActivationFunctionType.Relu,
                                bias=btile[:, bslot, jt:jt + 1],
                            )
                    if last:
                        # ship this jg's four feature blocks via SWDGE
                        # (GpSimd) so the store never head-of-line
                        # blocks the paced W chain.  Last jg goes out
                        # per (chunk, block) so the tail DMA is small.
                        if jg < JG - 1:
                            nc.gpsimd.dma_start(
                                outv[:, jg * 4:(jg + 1) * 4, :],
                                dst[:, jg * 4:(jg + 1) * 4, :])
                        else:
                            for c in range(2):
                                for jj in range(4):
                                    jt = jg * 4 + jj
                                    fin = (c == 1 and jj == 3)
                                    eng = nc.scalar if fin else nc.gpsimd
                                    eng.dma_start(outv[:, jt, csl[c]],
                                                  dst[:, jt, csl[c]])

    nc.compile()
    _cache[key] = nc
    return nc


def _linearize_w(W: np.ndarray) -> np.ndarray:
    """[D, D] -> bf16 [JG, 128, KT*512] with
    (jg, p, kt, jc) = W[kt*128+p, jg*512+jc]."""
    return np.ascontiguousarray(
        W.astype(BF16_NP).reshape(KT, 128, JG, 512)
        .transpose(2, 1, 0, 3).reshape(JG, 128, KT * 512))


def kernel(x, path_mask, W0, b0, W1, b1, W2, b2, W3, b3, _trace=False):
    x = np.asarray(x, dtype=np.float32)
    Wls = [np.asarray(W, dtype=np.float32) for W in (W0, W1, W2, W3)]
    bls = [np.asarray(b, dtype=np.float32) for b in (b0, b1, b2, b3)]
    B = x.shape[0]

    pm = np.asarray(path_mask)
    e3 = (pm[:, 0] * 4 + pm[:, 1] * 2 + pm[:, 2]).astype(np.int64)
    leaf_rows = [np.nonzero(e3 == e)[0] for e in range(8)]
    counts = np.array([len(r) for r in leaf_rows])

    # per-core chunk assignment: core 2j+k gets half k of leaf 2j in
    # chunk 0 and half k of leaf 2j+1 in chunk 1.
    halves = [None] * 8  # halves[leaf] = (rows_half0, rows_half1)
    for e in range(8):
        h = (counts[e] + 1) // 2
        halves[e] = (leaf_rows[e][:h], leaf_rows[e][h:])
    c0 = max(2, (max(len(halves[2 * j][0]) for j in range(4)) + 1) // 2 * 2)
    c1 = max(2, (max(len(halves[2 * j + 1][0]) for j in range(4)) + 1) // 2 * 2)
    # nseg > 1 only under extreme routing skew (a leaf with > 1024
    # rows); each extra segment re-runs the kernel on the overflow.
    nseg = max(1, math.ceil(c0 / 512), math.ceil(c1 / 512))
    c0 = min(c0, 512)
    c1 = min(c1, 512)
    Bc = c0 + c1
    nc = _build(c0, c1)

    xT_bf = np.ascontiguousarray(x.T.astype(BF16_NP))
    in_common = []
    for cid in range(N_CORES):
        j = cid // 2
        eids = (0, j >> 1, j)
        m = {f"W{l}": _linearize_w(Wls[l][eids[l]]) for l in range(3)}
        m["W3A"] = _linearize_w(Wls[3][2 * j])
        m["W3B"] = _linearize_w(Wls[3][2 * j + 1])
        m["bias"] = np.ascontiguousarray(np.stack(
            [bls[0][0], bls[1][j >> 1], bls[2][j],
             bls[3][2 * j], bls[3][2 * j + 1]]))
        in_common.append(m)

    core_groups = []
    for cid in range(N_CORES):
        j, k = cid // 2, cid % 2
        core_groups.append((halves[2 * j][k], halves[2 * j + 1][k]))

    out_full = np.zeros((B, D), dtype=np.float32)
    last_res = None
    for s in range(nseg):
        in_maps = []
        segs = []
        for cid in range(N_CORES):
            g0, g1 = core_groups[cid]
            g0 = g0[s * c0:(s + 1) * c0]
            g1 = g1[s * c1:(s + 1) * c1]
            segs.append((g0, g1))
            xTc = np.zeros((D, Bc), dtype=BF16_NP)
            xTc[:, :len(g0)] = xT_bf[:, g0]
            xTc[:, c0:c0 + len(g1)] = xT_bf[:, g1]
            # [D, Bc] -> SBUF-layout [128, KT*Bc]
            xlin = np.ascontiguousarray(
                xTc.reshape(KT, 128, Bc).transpose(1, 0, 2)
                .reshape(128, KT * Bc))
            in_maps.append({"xT": xlin, **in_common[cid]})
        res = run_bass_kernel_spmd(nc, in_maps, list(range(N_CORES)),
                                   trace=_trace)
        last_res = res
        for cid in range(N_CORES):
            g0, g1 = segs[cid]
            # SBUF-layout [128, JT*Bc] -> [D, Bc]
            o = res.results[cid]["out"].reshape(128, JT, Bc)
            o = o.transpose(1, 0, 2).reshape(D, Bc)
            out_full[g0] = o[:, :len(g0)].astype(np.float32).T
            out_full[g1] = o[:, c0:c0 + len(g1)].astype(np.float32).T
    if _trace:
        return out_full, last_res
    return out_full
